# revision 55
# baseline (speedup 1.0000x reference)
"""Trainium2 Bass kernel for a dense transformer block.

Data-parallel over batch B=8 across 8 NeuronCores (one batch element per
core, weights replicated, no collectives).

Per core (x_b is [T=1024, C=1024] fp32):
  h  = LN1(x);  per-head q,k,v = h @ Wq/Wk/Wv;  S = q k^T / 8 with the
  "staircase" mask, which is exactly block-causal at 64 granularity
  (row r attends to keys [0, (r//64+1)*64) );  out = softmax(S) v
  x2 = x + cat(out) @ Wo + bo;  y = x2 + relu(LN2(x2) @ W1 + b1) @ W2 + b2

Layout strategy (v2 — all matmul operands bf16, 1 cycle/row on the PE):
  - token-major tiles [128 tokens, C] for LN / residuals; channel-major
    activations (transposed on the PE) feed every matmul contraction
  - attention computes S^T [keys, queries] per head; exp(S^T) tiles are the
    MOVING operand of an A@V matmul whose stationary is [V | ones] [128, 65]
    — psum accumulates the unnormalized output [65, queries] with the
    softmax denominator in row 64.  The reciprocal row is DMA-broadcast
    across 64 partitions and multiplied in on the DVE; odd heads reach
    outT partitions 64..127 via a small sbuf->sbuf DMA shift.  This kills
    the 64 per-head output transposes and 384 tiny matmuls of v1.
  - weights are pre-cast to bf16 on the host (halves HBM traffic; FWL
    halves PE weight-load time).
"""

import os

import numpy as np
import ml_dtypes

import concourse.bass as bass
import concourse.mybir as mybir
import concourse.tile as tile
from concourse import bacc
from concourse.masks import make_identity
from concourse.bass_utils import run_bass_kernel_spmd

T, C, H, HS = 1024, 1024, 16, 64
NT = T // 128          # 8 token tiles
NCH = C // 128         # 8 channel chunks
NPAIR = H // 2         # 8 head pairs
FF = 4 * C             # 4096
NG = FF // 128         # 32 FFN hidden groups
EPS = 1e-5
F32 = mybir.dt.float32
BF16 = mybir.dt.bfloat16


def _layernorm(nc, pool, x_ap, out_ap, eps_tile, split=True):
    """LN along the free dim (C=1024) of a [128, 1024] tile -> bf16 out.

    split=True shares the normalize between DVE and Pool; split=False keeps
    it all on DVE (used in phase 1 so a busy Pool tail never gates it)."""
    stats = pool.tile([128, 2, 6], F32, tag="ln_stats", name="ln_stats")
    mv = pool.tile([128, 2], F32, tag="ln_mv", name="ln_mv")
    xr = x_ap.rearrange("p (s f) -> p s f", s=2)
    for s in range(2):
        nc.vector.bn_stats(out=stats[:, s, :], in_=xr[:, s, :])
    nc.vector.bn_aggr(out=mv, in_=stats)
    rstd = pool.tile([128, 1], F32, tag="ln_rstd", name="ln_rstd")
    nc.scalar.activation(
        out=rstd, in_=mv[:, 1:2],
        func=mybir.ActivationFunctionType.Sqrt,
        bias=eps_tile, scale=1.0,
    )
    nc.vector.reciprocal(out=rstd, in_=rstd)
    cut = 512 if split else 1024
    nc.vector.tensor_scalar(
        out=out_ap[:, 0:cut], in0=x_ap[:, 0:cut],
        scalar1=mv[:, 0:1], scalar2=rstd,
        op0=mybir.AluOpType.subtract, op1=mybir.AluOpType.mult,
    )
    if split:
        nc.gpsimd.tensor_scalar(
            out=out_ap[:, 512:1024], in0=x_ap[:, 512:1024],
            scalar1=mv[:, 0:1], scalar2=rstd,
            op0=mybir.AluOpType.subtract, op1=mybir.AluOpType.mult,
        )


def build_program():
    nc = bacc.Bacc("TRN2", target_bir_lowering=False, debug=False, num_devices=8)

    x_d = nc.dram_tensor("x", [T, C], F32, kind="ExternalInput").ap()
    wq_d = nc.dram_tensor("wq", [C, C], BF16, kind="ExternalInput").ap()
    wk_d = nc.dram_tensor("wk", [C, C], BF16, kind="ExternalInput").ap()
    wv_d = nc.dram_tensor("wv", [C, C], BF16, kind="ExternalInput").ap()
    wo_d = nc.dram_tensor("wo", [C, C], BF16, kind="ExternalInput").ap()
    w1_d = nc.dram_tensor("w1", [C, FF], BF16, kind="ExternalInput").ap()
    w2_d = nc.dram_tensor("w2", [FF, C], BF16, kind="ExternalInput").ap()
    bo_d = nc.dram_tensor("bo", [C], F32, kind="ExternalInput").ap()
    b1_d = nc.dram_tensor("b1", [FF], F32, kind="ExternalInput").ap()
    b2_d = nc.dram_tensor("b2", [C], F32, kind="ExternalInput").ap()
    ln1g_d = nc.dram_tensor("ln1g", [C], F32, kind="ExternalInput").ap()
    ln1b_d = nc.dram_tensor("ln1b", [C], F32, kind="ExternalInput").ap()
    ln2g_d = nc.dram_tensor("ln2g", [C], F32, kind="ExternalInput").ap()
    ln2b_d = nc.dram_tensor("ln2b", [C], F32, kind="ExternalInput").ap()
    y_d = nc.dram_tensor("y", [T, C], F32, kind="ExternalOutput").ap()

    reps = int(os.environ.get("KERNEL_REPS", "1"))
    with tile.TileContext(nc) as tc:
        # rep-invariant prelude: constants + replicated parameters
        pre = tc.alloc_tile_pool(name="prelude", bufs=1)
        c = _Prelude()
        c.ident = pre.tile([128, 128], BF16, name="ident")
        make_identity(nc, c.ident)
        c.eps = pre.tile([128, 1], F32, name="eps")
        nc.vector.memset(c.eps, EPS)
        c.b1_sb = pre.tile([128, NG], F32, name="b1_sb")
        nc.gpsimd.dma_start(out=c.b1_sb,
                            in_=b1_d.rearrange("(g p) -> p g", p=128))
        ones_f32 = pre.tile([1, 64], F32, name="ones_f32")
        nc.vector.memset(ones_f32, 1.0)
        c.ones_col = pre.tile([1, 64], mybir.dt.float32r, name="ones_col")
        nc.vector.tensor_copy(out=c.ones_col, in_=ones_f32)
        for nm, src in (("ln1g_c", ln1g_d), ("ln1b_c", ln1b_d),
                        ("ln2g_c", ln2g_d), ("ln2b_c", ln2b_d)):
            t = pre.tile([128, NCH], F32, name=nm)
            nc.gpsimd.dma_start(out=t, in_=src.rearrange("(j p) -> p j", p=128))
            setattr(c, nm, t)
        c.bo_r = _rep(nc, pre, "bo_r", bo_d, C)
        c.b2_r = _rep(nc, pre, "b2_r", b2_d, C)
        for _ in range(reps):
            _emit(nc, tc, c, x_d, wq_d, wk_d, wv_d, wo_d, w1_d, w2_d, y_d)
        pre.release()
    nc.compile()
    return nc


class _Prelude:
    pass


def _tr2(nc, c, pspool, h2T, h_prev, i_prev):
    """Transpose one LN2 output tile into h2T (g/b fused on ACT)."""
    for j in range(NCH):
        ps = pspool.tile([128, 128], BF16, tag="tr2", bufs=4, name="ps_tr2")
        nc.tensor.transpose(ps, h_prev[:, j * 128:(j + 1) * 128], c.ident)
        nc.scalar.activation(
            out=h2T[:, j, i_prev * 128:(i_prev + 1) * 128], in_=ps,
            func=mybir.ActivationFunctionType.Identity,
            bias=c.ln2b_c[:, j:j + 1], scale=c.ln2g_c[:, j:j + 1])


def _rep(nc, pool, name, src, n):
    """Replicate a [n] vector across 128 partitions."""
    t = pool.tile([128, n], F32, tag=name, name=name)
    nc.gpsimd.dma_start(out=t, in_=src.unsqueeze(0).to_broadcast((128, n)))
    return t


def _emit(nc, tc, c, x_d, wq_d, wk_d, wv_d, wo_d, w1_d, w2_d, y_d):
    ident, eps_tile, ones_col, b1_sb = c.ident, c.eps, c.ones_col, c.b1_sb

    ln_pool = tc.alloc_tile_pool(name="ln", bufs=3)

    # ---- Phase 1: LN1 + transpose to channel-major + V projection ----
    hT_pool = tc.alloc_tile_pool(name="hTp", bufs=1)
    hT = hT_pool.tile([128, NCH, T], BF16, name="hT")
    v_pool = tc.alloc_tile_pool(name="vAp", bufs=1)
    v_all = v_pool.tile([128, NT, H * 65], BF16, name="v_all")
    for hh in range(H):
        nc.gpsimd.memset(v_all[:, :, 65 * hh + 64:65 * hh + 65], 1.0)
    with tc.tile_pool(name="xin1", bufs=1, side="right") as x_pool, \
         tc.tile_pool(name="h", bufs=2) as h_pool, \
         tc.tile_pool(name="wvg", bufs=1, side="right") as wv_pool, \
         tc.tile_pool(name="p1ps", bufs=1, space="PSUM") as p1ps:
        x_sb = x_pool.tile([128, NT, C], F32, name="x_sb")
        for i in range(NT):
            # ACT-queue DMA: issues during the previous rep's FFN tail
            nc.scalar.dma_start(out=x_sb[:, i, :],
                                in_=x_d[i * 128:(i + 1) * 128, :])
        wv_t = wv_pool.tile([128, NCH, H * 64], BF16, name="wv_t")
        for grp in range(2):
            nc.scalar.dma_start(
                out=wv_t[:, :, grp * 512:(grp + 1) * 512],
                in_=wv_d[:, grp * 512:(grp + 1) * 512].rearrange(
                    "(ch cp) n -> cp ch n", cp=128))
        for i in range(NT):
            h_t = h_pool.tile([128, C], BF16, tag="h", name="h_t")
            _layernorm(nc, ln_pool, x_sb[:, i, :], h_t, eps_tile)
            for j in range(NCH):
                ps = p1ps.tile([128, 128], BF16, tag="tr", bufs=4, name="ps_tr")
                nc.tensor.transpose(ps, h_t[:, j * 128:(j + 1) * 128], ident)
                nc.vector.tensor_scalar(
                    out=hT[:, j, i * 128:(i + 1) * 128], in0=ps,
                    scalar1=c.ln1g_c[:, j:j + 1], scalar2=c.ln1b_c[:, j:j + 1],
                    op0=mybir.AluOpType.mult, op1=mybir.AluOpType.add)
            for grp in range(2):
                ps_v = p1ps.tile([128, 512], F32, tag="v", bufs=2, name="ps_v")
                for j in range(NCH):
                    nc.tensor.matmul(
                        ps_v, hT[:, j, i * 128:(i + 1) * 128],
                        wv_t[:, j, grp * 512:(grp + 1) * 512],
                        start=(j == 0), stop=(j == NCH - 1))
                # one strided copy per group: 8 heads x 64 dims at 65 stride
                v_dst = v_all[:, i, :].rearrange("p (h m) -> p h m", h=H)
                nc.scalar.activation(
                    out=v_dst[:, grp * 8:(grp + 1) * 8, 0:64],
                    in_=ps_v.rearrange("p (h m) -> p h m", h=8),
                    func=mybir.ActivationFunctionType.Copy)

    # ---- Phase 2: per head-pair QK + attention ----
    outT_pool = tc.alloc_tile_pool(name="outTp", bufs=1, side="right")
    outT = outT_pool.tile([128, NPAIR, T], BF16, name="outT")

    with tc.tile_pool(name="wqk", bufs=2) as w_pool, \
         tc.tile_pool(name="qk", bufs=2) as qk_pool, \
         tc.tile_pool(name="expS", bufs=8) as e_pool, \
         tc.tile_pool(name="rcp", bufs=2) as r_pool, \
         tc.tile_pool(name="attps", bufs=1, space="PSUM") as aps:
        for p in range(NPAIR):
            wq_t = w_pool.tile([128, NCH, 128], BF16, tag="wq", name="wq_t")
            wk_t = w_pool.tile([128, NCH, 128], BF16, tag="wk", name="wk_t")
            csl = slice(p * 128, (p + 1) * 128)
            for wt, wd in ((wq_t, wq_d), (wk_t, wk_d)):
                nc.sync.dma_start(
                    out=wt, in_=wd[:, csl].rearrange("(ch cp) n -> cp ch n", cp=128))

            qT = qk_pool.tile([128, T], BF16, tag="qT", name="qT")   # [2*HS, T]
            kT = qk_pool.tile([128, T], BF16, tag="kT", name="kT")
            for dst, wt in ((qT, wq_t), (kT, wk_t)):
                for half in range(2):
                    ps = aps.tile([128, 512], F32, tag="qkv", bufs=2, name="ps_qk")
                    for j in range(NCH):
                        nc.tensor.matmul(
                            ps, wt[:, j, :],
                            hT[:, j, half * 512:(half + 1) * 512],
                            start=(j == 0), stop=(j == NCH - 1))
                    nc.vector.tensor_copy(
                        out=dst[:, half * 512:(half + 1) * 512], in_=ps)

            # attention in query halves; S^T tiles are [keys, queries]
            for th in range(2):
                t0 = th * 512
                njt = (th + 1) * 4          # key tiles 0..njt-1 participate
                eS = [[None] * njt for _ in range(2)]
                for j in range(njt):
                    c0 = max(0, j * 128 - t0)   # first valid query col
                    for hh in range(2):
                        hsl = slice(hh * 64, (hh + 1) * 64)
                        ps = aps.tile([128, 512], F32, tag=f"sc{hh}", bufs=2,
                                      name="ps_sc")
                        nc.tensor.matmul(
                            ps[:, c0:512],
                            kT[hsl, j * 128:(j + 1) * 128],
                            qT[hsl, t0 + c0:t0 + 512],
                            start=True, stop=True,
                            tile_position=(hh * 64, 0))
                        et = e_pool.tile([128, 512], BF16, tag=f"e{hh}",
                                         name="eS_t")
                        nc.scalar.activation(
                            out=et[:, c0:512], in_=ps[:, c0:512],
                            func=mybir.ActivationFunctionType.Exp,
                            scale=float(HS) ** -0.5)
                        if j * 128 >= t0:   # diagonal tile: zero masked quadrant
                            nc.gpsimd.memset(et[64:128, c0:c0 + 64], 0.0)
                        eS[hh][j] = et
                # A@V: stationary [V | ones] [128, 65], moving exp(S^T).
                # psum rows 0..63 = unnormalized out, row 64 = denominator.
                psu = [None, None]
                for hh in range(2):
                    head = 2 * p + hh
                    psu[hh] = aps.tile([65, 512], F32, tag=f"u{hh}", bufs=1,
                                       name="ps_u")
                    for j in range(njt):
                        c0 = max(0, j * 128 - t0)
                        nc.tensor.matmul(
                            psu[hh][:, c0:512],
                            v_all[:, j, 65 * head:65 * head + 65],
                            eS[hh][j][:, c0:512],
                            start=(j == 0), stop=(j == njt - 1),
                            skip_group_check=True)
                rcp = r_pool.tile([1, 1024], mybir.dt.float32r, tag="rcp",
                                  name="rcp")
                rcp_sb = r_pool.tile([64, 2, 512], F32, tag="rcps", name="rcp_sb")
                stage1 = r_pool.tile([64, 512], BF16, tag="stg", name="stage1")
                for hh in range(2):
                    with nc.allow_low_precision(
                            reason="softmax denom reciprocal in f32r"):
                        nc.vector.reciprocal(
                            out=rcp[:, 512 * hh:512 * hh + 512],
                            in_=psu[hh][64:65, :])
                    ps_b = aps.tile([128, 512], F32, tag="sc0", bufs=2,
                                    name="ps_b")
                    nc.tensor.matmul(
                        ps_b[0:64, :],
                        ones_col,
                        rcp[0:1, 512 * hh:512 * hh + 512],
                        start=True, stop=True)
                    nc.scalar.activation(
                        out=rcp_sb[:, hh, :], in_=ps_b[0:64, :],
                        func=mybir.ActivationFunctionType.Copy)
                nc.vector.tensor_tensor(
                    out=stage1, in0=psu[1][0:64, :],
                    in1=rcp_sb[:, 1, :], op=mybir.AluOpType.mult)
                nc.sync.dma_start(
                    out=outT[64:128, p, t0:t0 + 512], in_=stage1)
                nc.vector.tensor_tensor(
                    out=outT[0:64, p, t0:t0 + 512], in0=psu[0][0:64, :],
                    in1=rcp_sb[:, 0, :], op=mybir.AluOpType.mult)
    v_pool.release()
    hT_pool.release()
    # preload the sqrt act-table during the attention tail so LN2's first
    # rstd doesn't pay the set switch
    warm_pool = tc.alloc_tile_pool(name="warm", bufs=1)
    warm = warm_pool.tile([1, 1], F32, name="warm")
    nc.scalar.activation(out=warm, in_=warm,
                         func=mybir.ActivationFunctionType.Sqrt)
    warm_pool.release()

    # ---- Phase 3+4 fused: output projection + residual + LN2 + transpose ----
    x2_pool = tc.alloc_tile_pool(name="x2p", bufs=1)
    x2 = x2_pool.tile([128, NT, C], BF16, name="x2")
    h2T_pool = tc.alloc_tile_pool(name="h2Tp", bufs=1)
    h2T = h2T_pool.tile([128, NCH, T], BF16, name="h2T")
    h2_pool = tc.alloc_tile_pool(name="h2", bufs=4)
    late_tr2 = []
    with tc.tile_pool(name="wo", bufs=1) as wo_pool, \
         tc.tile_pool(name="xin2", bufs=3) as x_pool, \
         tc.tile_pool(name="prps", bufs=1, space="PSUM") as prps:
        wo_t = wo_pool.tile([128, NCH, C], BF16, name="wo_t")
        for ch in range(NCH):
            nc.sync.dma_start(
                out=wo_t[:, ch, :], in_=wo_d[ch * 128:(ch + 1) * 128, :])
        pend = None
        for i in range(NT):
            x_t = x_pool.tile([128, C], F32, tag="x", name="x_t2")
            nc.sync.dma_start(out=x_t, in_=x_d[i * 128:(i + 1) * 128, :])
            # x + bo on Pool, off the Wo-psum critical path
            xbo = x_pool.tile([128, C], F32, tag="xbo", name="xbo")
            nc.gpsimd.tensor_add(out=xbo, in0=x_t, in1=c.bo_r)
            for half in range(2):
                ps = prps.tile([128, 512], F32, tag="pr", bufs=3, name="ps_pr")
                for ch in range(NCH):
                    nc.tensor.matmul(
                        ps, outT[:, ch, i * 128:(i + 1) * 128],
                        wo_t[:, ch, half * 512:(half + 1) * 512],
                        start=(ch == 0), stop=(ch == NCH - 1))
                hsl = slice(half * 512, (half + 1) * 512)
                nc.vector.tensor_add(
                    out=x2[:, i, hsl], in0=ps, in1=xbo[:, hsl])
            h_t = h2_pool.tile([128, C], BF16, tag="h2", name="h2_t")
            _layernorm(nc, ln_pool, x2[:, i, :], h_t, eps_tile)
            # transposes for tile i-1: their LN2 finished during Wo(i).
            # tiles 4..7 (needed only by W1's second half) are deferred into
            # the FFN W1 loop so the last tile's LN2 latency is covered.
            if pend is not None:
                if pend[1] <= 3:
                    _tr2(nc, c, prps, h2T, *pend)
                else:
                    late_tr2.append(pend)
            pend = (h_t, i)
        late_tr2.append(pend)
    outT_pool.release()

    # ---- Phase 5: FFN in t-halves ----
    late_jobs = [(hp, ip, j) for hp, ip in late_tr2 for j in range(NCH)]
    with tc.tile_pool(name="w1", bufs=8) as w1_pool, \
         tc.tile_pool(name="w2", bufs=12) as w2_pool, \
         tc.tile_pool(name="uTp", bufs=1) as uT_pool, \
         tc.tile_pool(name="yout", bufs=2) as out_pool, \
         tc.tile_pool(name="ups", bufs=2, space="PSUM") as ups, \
         tc.tile_pool(name="fps", bufs=1, space="PSUM") as fps:
        for th in range(2):
            t0 = th * 512
            uT = uT_pool.tile([128, NG, 512], BF16, tag="uT", bufs=2, name="uT")
            for g in range(NG):
                w1_t = w1_pool.tile([128, NCH, 128], BF16, tag="w1", name="w1_t")
                nc.sync.dma_start(
                    out=w1_t,
                    in_=w1_d[:, g * 128:(g + 1) * 128].rearrange(
                        "(ch cp) n -> cp ch n", cp=128))
                ps = ups.tile([128, 512], F32, tag="u", name="ps_u")
                for j in range(NCH):
                    nc.tensor.matmul(
                        ps, w1_t[:, j, :],
                        h2T[:, j, t0:t0 + 512],
                        start=(j == 0), stop=(j == NCH - 1))
                nc.scalar.activation(
                    out=uT[:, g, :], in_=ps,
                    func=mybir.ActivationFunctionType.Relu,
                    bias=b1_sb[:, g:g + 1], scale=1.0)
                # deferred LN2 transposes (tiles 4..7), spread through th=0
                if th == 0 and g >= 4:
                    for _ in range(2):
                        if late_jobs:
                            hp, ip, j = late_jobs.pop(0)
                            ps2 = ups.tile([128, 128], BF16, tag="tr2",
                                           bufs=2, name="ps_tr2l")
                            nc.tensor.transpose(
                                ps2, hp[:, j * 128:(j + 1) * 128], ident)
                            nc.scalar.activation(
                                out=h2T[:, j, ip * 128:(ip + 1) * 128],
                                in_=ps2,
                                func=mybir.ActivationFunctionType.Identity,
                                bias=c.ln2b_c[:, j:j + 1],
                                scale=c.ln2g_c[:, j:j + 1])
            for chh in range(2):
                hsl = slice(chh * 512, (chh + 1) * 512)
                ps_f = [fps.tile([128, 512], F32, tag=f"f{it}", name=f"ps_f{it}")
                        for it in range(4)]
                for k in range(NG):
                    w2_t = w2_pool.tile([128, 512], BF16, tag="w2", name="w2_t")
                    nc.sync.dma_start(
                        out=w2_t, in_=w2_d[k * 128:(k + 1) * 128, hsl])
                    for it in range(4):
                        nc.tensor.matmul(
                            ps_f[it],
                            uT[:, k, it * 128:(it + 1) * 128],
                            w2_t,
                            start=(k == 0), stop=(k == NG - 1))
                for it in range(4):
                    gi = th * 4 + it
                    o_t = out_pool.tile([128, 512], F32, tag="y", name="y_t")
                    nc.vector.tensor_add(
                        out=o_t, in0=ps_f[it], in1=x2[:, gi, hsl])
                    nc.vector.tensor_add(out=o_t, in0=o_t, in1=c.b2_r[:, hsl])
                    nc.gpsimd.dma_start(
                        out=y_d[gi * 128:(gi + 1) * 128, hsl], in_=o_t)
    h2_pool.release()
    h2T_pool.release()
    x2_pool.release()
    ln_pool.release()


_NC_CACHE = {}


def _get_program():
    if "nc" not in _NC_CACHE:
        _NC_CACHE["nc"] = build_program()
    return _NC_CACHE["nc"]


def _prep_inputs(x, Wq, Wk, Wv, Wo, bo, ln1_g, ln1_b, ln2_g, ln2_b, W1, b1, W2, b2):
    f = lambda a: np.ascontiguousarray(np.asarray(a, dtype=np.float32))
    bf = lambda a: np.ascontiguousarray(
        np.asarray(a, dtype=np.float32).astype(ml_dtypes.bfloat16))
    wq2 = np.asarray(Wq, np.float32).transpose(1, 0, 2).reshape(C, C)
    wk2 = np.asarray(Wk, np.float32).transpose(1, 0, 2).reshape(C, C)
    wv2 = np.asarray(Wv, np.float32).transpose(1, 0, 2).reshape(C, C)
    return {
        "wq": bf(wq2), "wk": bf(wk2), "wv": bf(wv2), "wo": bf(Wo),
        "w1": bf(W1), "w2": bf(W2),
        "bo": f(bo), "b1": f(b1), "b2": f(b2),
        "ln1g": f(ln1_g), "ln1b": f(ln1_b), "ln2g": f(ln2_g), "ln2b": f(ln2_b),
    }


def kernel(x, mask, Wq, Wk, Wv, Wo, bo, ln1_g, ln1_b, ln2_g, ln2_b, W1, b1, W2, b2):
    x = np.ascontiguousarray(np.asarray(x, dtype=np.float32))
    B = x.shape[0]
    common = _prep_inputs(x, Wq, Wk, Wv, Wo, bo, ln1_g, ln1_b,
                          ln2_g, ln2_b, W1, b1, W2, b2)
    nc = _get_program()
    in_maps = [dict(common, x=np.ascontiguousarray(x[b])) for b in range(B)]
    res = run_bass_kernel_spmd(nc, in_maps, list(range(B)))
    return np.stack([res.results[b]["y"] for b in range(B)], axis=0)
